# revision 1
# baseline (speedup 1.0000x reference)
"""BERT self-attention (B=4, S=2048, D=1024, H=16) on 8 Trainium2 NeuronCores.

Tensor-parallel (Megatron) over heads: core c owns heads 2c, 2c+1.
  - Wq/Wk/Wv column-sharded (128 output dims per core), Wo row-sharded.
  - Each core consumes the full x, produces a partial (8192, 1024) output;
    partials are summed on the host (the Wo contraction over d_model is
    split across cores), plus bo.

Per-core dataflow (all big matmuls in fp32r: 11-bit-mantissa inputs,
fp32 accumulate — full PE rate at free-dim >= 256):
  xT (1024, 8192) streamed in 512-token blocks
    -> Q,K (dq 128, tok 8192) dk-major   [lhsT=WqT/WkT k-tiles, rhs=xT]
    -> V (dv 128, tok 8192), PE-transposed per 128-tok tile into
       vt [tok 128, 130] = [Vh0 64 | ones | Vh1 64 | ones]
  scores.T tile [ktok 128, q 1024] = both heads' [*, 512] halves
    (row-packed K=64 matmul pair into the two PSUM banks of one tile)
  exp on ScalarE (scale=1/8 (+ mask bias per ktok partition if needed))
  ctx.T accumulation over 16 ktok tiles: lhsT=vt[:, h*65:(h+1)*65]
    (M=65: row 64 accumulates the softmax denominators for free)
  normalize: reciprocal of row 64 -> PE outer-product broadcast -> DVE mul
  out partial [tok 128, 512] = lhsT=ctxn tok-tile, rhs=WoT
"""
import sys

if "/opt/trn_rl_repo" not in sys.path:
    sys.path.insert(0, "/opt/trn_rl_repo")

import numpy as np

import concourse.bacc as bacc
import concourse.mybir as mybir
import concourse.tile as tile
from concourse.bass_utils import run_bass_kernel_spmd

DT = mybir.dt
AF = mybir.ActivationFunctionType

B, S, D, H = 4, 2048, 1024, 16
DK = D // H  # 64
NCORES = 8
HPC = H // NCORES  # heads per core = 2
DPC = HPC * DK  # output dims per core = 128
T = B * S  # 8192 tokens
TB = 512  # token block for projections
QB = 512  # query block for attention
NKT = S // 128  # 16 key tiles per sequence
NDT = D // 128  # 8 contraction tiles for projections

_cache = {}


def _build(with_mask, phase="full", nb=B, reps=1):
    nc = bacc.Bacc("TRN2", target_bir_lowering=False, debug=False)
    xT_d = nc.declare_dram_parameter("xT", [D, T], DT.float32r, isOutput=False)
    wq_d = nc.declare_dram_parameter("wqT", [D, DPC], DT.float32r, isOutput=False)
    wk_d = nc.declare_dram_parameter("wkT", [D, DPC], DT.float32r, isOutput=False)
    wv_d = nc.declare_dram_parameter("wvT", [D, DPC], DT.float32r, isOutput=False)
    wo_d = nc.declare_dram_parameter("woT", [DPC, D], DT.float32r, isOutput=False)
    bq_d = nc.declare_dram_parameter("bq", [DPC, 1], DT.float32, isOutput=False)
    bk_d = nc.declare_dram_parameter("bk", [DPC, 1], DT.float32, isOutput=False)
    bv_d = nc.declare_dram_parameter("bv", [DPC, 1], DT.float32, isOutput=False)
    id_d = nc.declare_dram_parameter("ident", [128, 128], DT.float32, isOutput=False)
    if with_mask:
        mb_d = nc.declare_dram_parameter("mbias", [B, NKT, 128], DT.float32, isOutput=False)
    out_d = nc.declare_dram_parameter("out", [T, D], DT.float32, isOutput=True)
    if phase == "qkv":
        dbg_d = nc.declare_dram_parameter("dbg", [3, 128, T], DT.float32, isOutput=True)

    with tile.TileContext(nc) as tc:
        with (
            tc.tile_pool(name="cst", bufs=1) as cst,
            tc.tile_pool(name="qkv", bufs=1) as qkv,
            tc.tile_pool(name="xt", bufs=10) as xtp,
            tc.tile_pool(name="vt", bufs=32) as vtp,
            tc.tile_pool(name="es", bufs=3) as esp,
            tc.tile_pool(name="cn", bufs=3) as cnp,
            tc.tile_pool(name="os", bufs=3) as osp,
            tc.tile_pool(name="sm", bufs=3) as smp,
            tc.tile_pool(name="sps", bufs=2, space="PSUM") as sps,
            tc.tile_pool(name="cps", bufs=2, space="PSUM") as cps,
            tc.tile_pool(name="pmm", bufs=2, space="PSUM") as pmm,
        ):
            # ---- constants / weights ----
            wq = cst.tile([128, NDT, DPC], DT.float32r, tag="wq")
            wk = cst.tile([128, NDT, DPC], DT.float32r, tag="wk")
            wv = cst.tile([128, NDT, DPC], DT.float32r, tag="wv")
            nc.sync.dma_start(wq[:], wq_d.rearrange("(a p) m -> p a m", p=128))
            nc.sync.dma_start(wk[:], wk_d.rearrange("(a p) m -> p a m", p=128))
            nc.sync.dma_start(wv[:], wv_d.rearrange("(a p) m -> p a m", p=128))
            wo = cst.tile([DPC, D], DT.float32r, tag="wo")
            nc.sync.dma_start(wo[:], wo_d[:])
            bq = cst.tile([DPC, 1], DT.float32, tag="bq")
            bk = cst.tile([DPC, 1], DT.float32, tag="bk")
            bv = cst.tile([DPC, 1], DT.float32, tag="bv")
            nc.sync.dma_start(bq[:], bq_d[:])
            nc.sync.dma_start(bk[:], bk_d[:])
            nc.sync.dma_start(bv[:], bv_d[:])
            ident = cst.tile([128, 128], DT.float32, tag="ident")
            nc.sync.dma_start(ident[:], id_d[:])
            ones128 = cst.tile([128, 1], DT.float32, tag="ones128")
            nc.vector.memset(ones128[:], 1.0)
            onesc_f = cst.tile([1, 64], DT.float32, tag="onescf")
            nc.vector.memset(onesc_f[:], 1.0)
            onesc = cst.tile([1, 64], DT.float32r, tag="onesc")
            nc.vector.tensor_copy(onesc[:], onesc_f[:])
            if with_mask:
                mb = cst.tile([128, B, NKT], DT.float32, tag="mb")
                nc.sync.dma_start(mb[:], mb_d.rearrange("b a p -> p b a"))

            # persistent activations (dk/dv-major), one tile per batch so
            # attention for batch b only depends on batch b's projections
            q_sb = [qkv.tile([128, S], DT.float32r, tag=f"q{b}", name=f"q{b}") for b in range(B)]
            k_sb = [qkv.tile([128, S], DT.float32r, tag=f"k{b}", name=f"k{b}") for b in range(B)]
            v_sb = [qkv.tile([128, S], DT.float32, tag=f"v{b}", name=f"v{b}") for b in range(B)]

            for rep in range(reps):
                # ---- QKV projections ----
                for tb in range(T // TB):
                    xts = []
                    for dt_i in range(NDT):
                        xt = xtp.tile([128, TB], DT.float32r, tag="xt", name=f"{rep}_xt{tb}_{dt_i}")
                        nc.sync.dma_start(
                            xt[:],
                            xT_d[dt_i * 128 : (dt_i + 1) * 128, tb * TB : (tb + 1) * TB],
                        )
                        xts.append(xt)
                    for pname, w, bias, dst in (
                        ("q", wq, bq, q_sb),
                        ("k", wk, bk, k_sb),
                        ("v", wv, bv, v_sb),
                    ):
                        acc = pmm.tile([128, TB], DT.float32, tag="pmm", name=f"{rep}_p{pname}{tb}")
                        for dt_i in range(NDT):
                            nc.tensor.matmul(
                                acc[:],
                                w[:, dt_i, :],
                                xts[dt_i][:],
                                start=(dt_i == 0),
                                stop=(dt_i == NDT - 1),
                            )
                        bq_i, bc_i = tb // (S // TB), tb % (S // TB)
                        nc.vector.tensor_scalar_add(
                            dst[bq_i][:, bc_i * TB : (bc_i + 1) * TB], acc[:], bias[:]
                        )

                if phase == "qkv":
                    for i, src in enumerate((q_sb, k_sb, v_sb)):
                        for bb in range(B):
                            nc.sync.dma_start(
                                dbg_d[i, :, bb * S : (bb + 1) * S],
                                src[bb][:].bitcast(DT.float32))
                # ---- attention per batch ----
                for b in range(B if phase == "full" else (nb if phase == "attn" else 0)):
                    base = 0
                    qsb, ksb, vsb = q_sb[b], k_sb[b], v_sb[b]
                    # V transpose: vt[kt] = [tok 128, 130] fp32r
                    vts = []
                    for kt in range(NKT):
                        vp = pmm.tile([128, 128], DT.float32, tag="pmm", name=f"{rep}_vp{b}_{kt}")
                        nc.tensor.transpose(
                            vp[:], vsb[:, base + kt * 128 : base + (kt + 1) * 128], ident[:]
                        )
                        vt = vtp.tile([128, 130], DT.float32r, tag="vt", name=f"{rep}_vt{b}_{kt}")
                        nc.vector.tensor_copy(vt[:, 0:64], vp[:, 0:64])
                        nc.vector.tensor_copy(vt[:, 65:129], vp[:, 64:128])
                        nc.vector.tensor_copy(vt[:, 64:65], ones128[:])
                        nc.vector.tensor_copy(vt[:, 129:130], ones128[:])
                        vts.append(vt)

                    for qb in range(S // QB):
                        qoff = qb * QB
                        out_row = b * S + qb * QB
                        cps_h = [
                            cps.tile([65, QB], DT.float32, tag="ctx", name=f"{rep}_c{b}_{qb}_{h}")
                            for h in range(2)
                        ]
                        for kt in range(NKT):
                            sp = sps.tile([128, 2 * QB], DT.float32, tag="sps", name=f"{rep}_s{b}_{qb}_{kt}")
                            for h in range(2):
                                hp = slice(h * 64, (h + 1) * 64)
                                nc.tensor.matmul(
                                    sp[:, h * QB : (h + 1) * QB],
                                    ksb[hp, base + kt * 128 : base + (kt + 1) * 128],
                                    qsb[hp, qoff : qoff + QB],
                                    start=True,
                                    stop=True,
                                )
                            es = esp.tile([128, 2 * QB], DT.float32r, tag="es", name=f"{rep}_e{b}_{qb}_{kt}")
                            ebias = mb[:, b, kt : kt + 1] if with_mask else 0.0
                            for h in range(2):
                                hs = slice(h * QB, (h + 1) * QB)
                                nc.scalar.activation(
                                    es[:, hs], sp[:, hs], AF.Exp, bias=ebias, scale=0.125
                                )
                                nc.tensor.matmul(
                                    cps_h[h][:],
                                    vts[kt][:, h * 65 : (h + 1) * 65],
                                    es[:, hs],
                                    start=(kt == 0),
                                    stop=(kt == NKT - 1),
                                )
                        # normalize -> ctxn [128 dv, QB] fp32r. Copy ctx PSUM
                        # out via one DVE op per head first, so the bank frees
                        # for the next q-block without waiting on the whole
                        # recip -> broadcast -> mul chain.
                        ctxn = cnp.tile([128, QB], DT.float32r, tag="cn", name=f"{rep}_n{b}_{qb}")
                        for h in range(2):
                            cs = smp.tile([65, QB], DT.float32, tag="cs", name=f"{rep}_cs{b}_{qb}_{h}")
                            nc.vector.tensor_copy(cs[:], cps_h[h][:])
                            rr = smp.tile([1, QB], DT.float32r, tag="rr", name=f"{rep}_r{b}_{qb}_{h}")
                            with nc.allow_low_precision(reason="softmax reciprocal fp32r"):
                                nc.vector.reciprocal(rr[:], cs[64:65, :])
                            bc = pmm.tile([64, QB], DT.float32, tag="pmm", name=f"{rep}_bc{b}_{qb}_{h}")
                            nc.tensor.matmul(bc[:], onesc[:], rr[:], start=True, stop=True)
                            with nc.allow_low_precision(reason="ctx normalize to fp32r"):
                                nc.vector.tensor_mul(
                                    ctxn[h * 64 : (h + 1) * 64, :], cs[0:64, :], bc[:]
                                )
                        # output projection for this q block
                        for tt in range(QB // 128):
                            for ob in range(2):
                                op = pmm.tile(
                                    [128, 512], DT.float32, tag="pmm", name=f"{rep}_o{b}_{qb}_{tt}_{ob}"
                                )
                                nc.tensor.matmul(
                                    op[:],
                                    ctxn[:, tt * 128 : (tt + 1) * 128],
                                    wo[:, ob * 512 : (ob + 1) * 512],
                                    start=True,
                                    stop=True,
                                )
                                ost = osp.tile([128, 512], DT.float32, tag="os", name=f"{rep}_q{b}_{qb}_{tt}_{ob}")
                                nc.scalar.activation(ost[:], op[:], AF.Copy)
                                nc.sync.dma_start(
                                    out_d[
                                        out_row + tt * 128 : out_row + (tt + 1) * 128,
                                        ob * 512 : (ob + 1) * 512,
                                    ],
                                    ost[:],
                                )
    nc.compile()
    return nc


def _make_runner(nc):
    """jit-compiled shard-mapped executor over the 8 cores, no donation so
    device-resident inputs can be reused across timed calls."""
    import jax
    from jax.experimental.shard_map import shard_map
    from jax.sharding import Mesh, NamedSharding, PartitionSpec

    from concourse import bass2jax as b2j

    b2j.install_neuronx_cc_hook()
    partition_name = nc.partition_id_tensor.name if nc.partition_id_tensor else None
    in_names, out_names, out_avals = [], [], []
    for alloc in nc.m.functions[0].allocations:
        if not isinstance(alloc, mybir.MemoryLocationSet):
            continue
        name = alloc.memorylocations[0].name
        if alloc.kind == "ExternalInput":
            if name != partition_name:
                in_names.append(name)
        elif alloc.kind == "ExternalOutput":
            out_names.append(name)
            out_avals.append(
                jax.core.ShapedArray(tuple(alloc.tensor_shape), DT.np(alloc.dtype))
            )
    n_params = len(in_names)
    all_in_names = list(in_names + out_names)
    if partition_name is not None:
        all_in_names.append(partition_name)

    def _body(*args):
        operands = list(args)
        if partition_name is not None:
            operands.append(b2j.partition_id_tensor())
        outs = b2j._bass_exec_p.bind(
            *operands,
            out_avals=tuple(out_avals),
            in_names=tuple(all_in_names),
            out_names=tuple(out_names),
            lowering_input_output_aliases=(),
            sim_require_finite=True,
            sim_require_nnan=True,
            nc=nc,
        )
        return tuple(outs)

    devices = jax.devices()[:NCORES]
    mesh = Mesh(np.asarray(devices), ("core",))
    spec = PartitionSpec("core")
    n_outs = len(out_names)
    fn = jax.jit(
        shard_map(
            _body,
            mesh=mesh,
            in_specs=(spec,) * (n_params + n_outs),
            out_specs=(spec,) * n_outs,
            check_rep=False,
        ),
        keep_unused=True,
    )

    def _body_chain(n):
        def run(*args):
            ins = args[:n_params]
            outs = tuple(args[n_params:])
            for _ in range(n):
                outs = _body(*ins, *outs)
            return outs

        return run

    def chain_fn(n):
        return jax.jit(
            shard_map(
                _body_chain(n),
                mesh=mesh,
                in_specs=(spec,) * (n_params + n_outs),
                out_specs=(spec,) * n_outs,
                check_rep=False,
            ),
            keep_unused=True,
        )

    sharding = NamedSharding(mesh, spec)

    def put(in_maps):
        concat = [
            np.concatenate([np.asarray(m[name]) for m in in_maps], axis=0)
            for name in in_names
        ]
        zeros = [
            np.zeros((NCORES * a.shape[0], *a.shape[1:]), a.dtype) for a in out_avals
        ]
        return [jax.device_put(a, sharding) for a in (*concat, *zeros)]

    fn.chain_fn = chain_fn
    return fn, put, out_names, out_avals


def _in_maps(x, attention_mask, Wq, bq, Wk, bk, Wv, bv, Wo, with_mask):
    x = np.ascontiguousarray(np.asarray(x, dtype=np.float32))
    xT = np.ascontiguousarray(x.reshape(T, D).T)  # (D, T)
    ident = np.eye(128, dtype=np.float32)
    in_maps = []
    for c in range(NCORES):
        r = slice(c * DPC, (c + 1) * DPC)
        m = {
            "xT": xT,
            "wqT": np.ascontiguousarray(np.asarray(Wq, np.float32)[r, :].T),
            "wkT": np.ascontiguousarray(np.asarray(Wk, np.float32)[r, :].T),
            "wvT": np.ascontiguousarray(np.asarray(Wv, np.float32)[r, :].T),
            "woT": np.ascontiguousarray(np.asarray(Wo, np.float32)[:, r].T),
            "bq": np.asarray(bq, np.float32)[r].reshape(DPC, 1),
            "bk": np.asarray(bk, np.float32)[r].reshape(DPC, 1),
            "bv": np.asarray(bv, np.float32)[r].reshape(DPC, 1),
            "ident": ident,
        }
        if with_mask:
            mask = np.asarray(attention_mask)
            mbias = np.where(mask == 0, np.float32(-1e30), np.float32(0.0)).astype(
                np.float32
            )
            m["mbias"] = np.ascontiguousarray(mbias.reshape(B, NKT, 128))
        in_maps.append(m)
    return in_maps


def _prepare(x, attention_mask, Wq, bq, Wk, bk, Wv, bv, Wo, bo):
    """Build (cached), upload inputs, return (fn, dev_args, out_names)."""
    mask = np.asarray(attention_mask)
    with_mask = not bool((mask != 0).all())
    key = ("runner", with_mask)
    if key not in _cache:
        nc = _build(with_mask)
        _cache[key] = _make_runner(nc)
    fn, put, out_names, out_avals = _cache[key]
    dev_args = put(
        _in_maps(x, attention_mask, Wq, bq, Wk, bk, Wv, bv, Wo, with_mask)
    )
    return fn, dev_args, out_names


def kernel(x, attention_mask, Wq, bq, Wk, bk, Wv, bv, Wo, bo):
    fn, dev_args, out_names = _prepare(
        x, attention_mask, Wq, bq, Wk, bk, Wv, bv, Wo, bo
    )
    outs = fn(*dev_args)
    out_global = np.asarray(outs[out_names.index("out")])  # (8*T, D)
    acc = out_global.reshape(NCORES, T, D).sum(axis=0, dtype=np.float32)
    acc += np.asarray(bo, np.float32)[None, :]
    return acc.reshape(B, S, D)



# revision 18
# speedup vs baseline: 28.1424x; 28.1424x over previous
"""BERT self-attention (B=4, S=2048, D=1024, H=16) on 8 Trainium2 NeuronCores.

Tensor-parallel (Megatron) over heads: core c owns heads 2c, 2c+1.
  - Wq/Wk/Wv column-sharded (128 output dims per core), Wo row-sharded.
  - Each core consumes the full x, produces a partial (8192, 1024) bf16
    output; partials are summed on the host (the Wo contraction over d_model
    is split across cores), plus bo.

Per-core dataflow, software-pipelined at instruction granularity:
  - The attention kt loop (scores pair -> one exp -> AV pair) is the
    steady-state spine; its ScalarE exp (~1.04us per kt tile) is the rate
    limiter, so every other PE work item is spliced into the loop as
    "fillers" that execute in the PE's exp-wait gaps:
      * next batch's QKV projection matmuls (2-MM chunks) + PSUM evictions
      * next batch's V transposes (PE transpose + DVE reassembly into
        vt [tok 128, 130] bf16 = [Vh0 64 | ones | Vh1 64 | ones])
      * previous q-block's normalize + output projection, delayed until its
        DVE/Pool reduction chain has certainly completed, so the out matmuls
        never sit in the PE's 4-deep wait queue blocking the spine.
  - scores.T tile [ktok 128, q 1024]: K=64 matmul pair (base partitions
    0/64) into the two PSUM banks of one tile; one Exp op on ScalarE
    (scale=1/8, optional mask bias per ktok partition) -> es bf16.
  - ctx.T accumulation over 16 kt tiles, lhsT=vt[:, h*65:(h+1)*65]
    (M=65: row 64 accumulates the softmax denominators for free).
  - normalize: DVE reciprocal of the den row -> GPSIMD partition_broadcast
    -> DVE multiply (no PE/PSUM involvement).
  - out partial [tok 128, 512] fp32 psum -> GPSIMD eviction into a
    per-q-block bf16 staging tile [128, 4, 1024] -> one DVE-issued DMA.
  - Engine roles: PE matmuls; ACT exp only; DVE QKV/ctx evictions +
    reciprocal + normalize + out DMA issue; GPSIMD (Pool) out evictions +
    partition broadcasts; SP input prefetch only.
"""
import sys

if "/opt/trn_rl_repo" not in sys.path:
    sys.path.insert(0, "/opt/trn_rl_repo")

import numpy as np

import concourse.bacc as bacc
import concourse.mybir as mybir
import concourse.tile as tile

DT = mybir.dt
AF = mybir.ActivationFunctionType

B, S, D, H = 4, 2048, 1024, 16
DK = D // H  # 64
NCORES = 8
HPC = H // NCORES  # heads per core = 2
DPC = HPC * DK  # output dims per core = 128
T = B * S  # 8192 tokens
TB = 512  # token block for projections
QB = 512  # query block for attention
NKT = S // 128  # 16 key tiles per sequence
NDT = D // 128  # 8 contraction tiles for projections

_cache = {}


def _build(with_mask, reps=1, with_bias=True):
    nc = bacc.Bacc("TRN2", target_bir_lowering=False, debug=False)
    xT_d = nc.declare_dram_parameter("xT", [D, T], DT.bfloat16, isOutput=False)
    wq_d = nc.declare_dram_parameter("wqT", [D, DPC], DT.bfloat16, isOutput=False)
    wk_d = nc.declare_dram_parameter("wkT", [D, DPC], DT.bfloat16, isOutput=False)
    wv_d = nc.declare_dram_parameter("wvT", [D, DPC], DT.bfloat16, isOutput=False)
    wo_d = nc.declare_dram_parameter("woT", [DPC, D], DT.float32r, isOutput=False)
    if with_bias:
        bq_d = nc.declare_dram_parameter("bq", [DPC, 1], DT.float32, isOutput=False)
        bk_d = nc.declare_dram_parameter("bk", [DPC, 1], DT.float32, isOutput=False)
        bv_d = nc.declare_dram_parameter("bv", [DPC, 1], DT.float32, isOutput=False)
    id_d = nc.declare_dram_parameter("ident", [128, 128], DT.bfloat16, isOutput=False)
    if with_mask:
        mb_d = nc.declare_dram_parameter("mbias", [B, NKT, 128], DT.float32, isOutput=False)
    out_d = nc.declare_dram_parameter("out", [T, D], DT.bfloat16, isOutput=True)

    with tile.TileContext(nc) as tc:
        with (
            tc.tile_pool(name="cst", bufs=1) as cst,
            tc.tile_pool(name="qkv", bufs=1) as qkv,
            tc.tile_pool(name="xt", bufs=20) as xtp,
            tc.tile_pool(name="vt", bufs=32) as vtp,
            tc.tile_pool(name="es", bufs=3) as esp,
            tc.tile_pool(name="cn", bufs=3) as cnp,
            tc.tile_pool(name="os", bufs=2) as osp,
            tc.tile_pool(name="sm", bufs=4) as smp,
            tc.tile_pool(name="sps", bufs=2, space="PSUM") as sps,
            tc.tile_pool(name="cps", bufs=2, space="PSUM") as cps,
            tc.tile_pool(name="qac", bufs=1, space="PSUM") as qac,
            tc.tile_pool(name="pmm", bufs=1, space="PSUM") as pmm,
        ):
            # ---- constants / weights ----
            wq = cst.tile([128, NDT, DPC], DT.bfloat16, tag="wq")
            wk = cst.tile([128, NDT, DPC], DT.bfloat16, tag="wk")
            wv = cst.tile([128, NDT, DPC], DT.bfloat16, tag="wv")
            nc.sync.dma_start(wq[:], wq_d.rearrange("(a p) m -> p a m", p=128))
            nc.sync.dma_start(wk[:], wk_d.rearrange("(a p) m -> p a m", p=128))
            nc.sync.dma_start(wv[:], wv_d.rearrange("(a p) m -> p a m", p=128))
            wo = cst.tile([DPC, D], DT.float32r, tag="wo")
            nc.sync.dma_start(wo[:], wo_d[:])
            if with_bias:
                bq = cst.tile([DPC, 1], DT.float32, tag="bq")
                bk = cst.tile([DPC, 1], DT.float32, tag="bk")
                bv = cst.tile([DPC, 1], DT.float32, tag="bv")
                nc.sync.dma_start(bq[:], bq_d[:])
                nc.sync.dma_start(bk[:], bk_d[:])
                nc.sync.dma_start(bv[:], bv_d[:])
            else:
                bq = bk = bv = None
            ident = cst.tile([128, 128], DT.bfloat16, tag="ident")
            nc.sync.dma_start(ident[:], id_d[:])
            ones128 = cst.tile([128, 1], DT.float32, tag="ones128")
            nc.vector.memset(ones128[:], 1.0)
            if with_mask:
                mb = cst.tile([128, B, NKT], DT.float32, tag="mb")
                nc.sync.dma_start(mb[:], mb_d.rearrange("b a p -> p b a"))

            # persistent activations (dk/dv-major), one tile per batch
            q_sb = [qkv.tile([128, S], DT.bfloat16, tag=f"q{b}", name=f"q{b}") for b in range(B)]
            k_sb = [qkv.tile([128, S], DT.bfloat16, tag=f"k{b}", name=f"k{b}") for b in range(B)]
            v_sb = [qkv.tile([128, S], DT.bfloat16, tag=f"v{b}", name=f"v{b}") for b in range(B)]
            pbias = (wq, wk, wv), (bq, bk, bv), (q_sb, k_sb, v_sb)

            def gen_qkv(rep, b):
                """Generator: emits batch b's QKV projections in small PE
                chunks. Yields after each ~2-matmul item."""
                for bc_i in range(S // TB):
                    tb = b * (S // TB) + bc_i
                    xts = []
                    for dt_i in range(NDT):
                        xt = xtp.tile([128, TB], DT.bfloat16, tag="xt",
                                      name=f"{rep}_xt{tb}_{dt_i}")
                        nc.sync.dma_start(
                            xt[:],
                            xT_d[dt_i * 128 : (dt_i + 1) * 128,
                                 tb * TB : (tb + 1) * TB],
                        )
                        xts.append(xt)
                    yield
                    for pi in range(3):
                        w, bias, dst = pbias[0][pi], pbias[1][pi], pbias[2][pi]
                        acc = qac.tile([128, TB], DT.float32, tag="qac",
                                       name=f"{rep}_p{pi}{tb}")
                        for dt_i in range(NDT):
                            nc.tensor.matmul(
                                acc[:], w[:, dt_i, :], xts[dt_i][:],
                                start=(dt_i == 0), stop=(dt_i == NDT - 1),
                            )
                            if dt_i % 2 == 1:
                                yield
                        dslice = dst[b][:, bc_i * TB : (bc_i + 1) * TB]
                        if with_bias:
                            nc.vector.tensor_scalar_add(dslice, acc[:], bias[:])
                        else:
                            nc.vector.tensor_copy(dslice, acc[:])
                        yield

            vts_all = {}

            def gen_vt(rep, b):
                """Generator: emits batch b's V transposes, one kt per item."""
                vsb = v_sb[b]
                vts = []
                vts_all[(rep, b)] = vts
                for kt in range(NKT):
                    vp = pmm.tile([128, 128], DT.bfloat16, tag="pmm",
                                  name=f"{rep}_vp{b}_{kt}")
                    nc.tensor.transpose(
                        vp[:], vsb[:, kt * 128 : (kt + 1) * 128], ident[:]
                    )
                    vt = vtp.tile([128, 130], DT.bfloat16, tag="vt",
                                  name=f"{rep}_vt{b}_{kt}")
                    nc.vector.tensor_copy(vt[:, 0:64], vp[:, 0:64])
                    nc.vector.tensor_copy(vt[:, 65:129], vp[:, 64:128])
                    nc.gpsimd.tensor_copy(vt[:, 64:65], ones128[:])
                    nc.gpsimd.tensor_copy(vt[:, 129:130], ones128[:])
                    vts.append(vt)
                    yield

            def finish_qb(rep, b, qb, ctxn):
                """Generator of PE items for a q-block's output projection;
                first item must only run once ctxn is ready (caller delays)."""
                out_row = b * S + qb * QB
                ost = osp.tile([128, QB // 128, D], DT.bfloat16, tag="os",
                               name=f"{rep}_os{b}_{qb}")
                for tt in range(QB // 128):
                    for ob in range(2):
                        op = pmm.tile([128, 512], DT.float32, tag="pmm",
                                      name=f"{rep}_o{b}_{qb}_{tt}_{ob}")
                        nc.tensor.matmul(
                            op[:],
                            ctxn[:, tt * 128 : (tt + 1) * 128],
                            wo[:, ob * 512 : (ob + 1) * 512],
                            start=True, stop=True,
                        )
                        # GPSIMD cannot read PSUM on HW; evict on DVE
                        nc.vector.tensor_copy(
                            ost[:, tt, ob * 512 : (ob + 1) * 512], op[:]
                        )
                        yield
                nc.sync.dma_start(
                    out_d[out_row : out_row + QB, :].rearrange(
                        "(tt p) d -> p tt d", p=128
                    ),
                    ost[:],
                )
                yield

            def pump(gens, n):
                """Advance the filler generator chain by up to n items."""
                while n > 0 and gens:
                    try:
                        next(gens[0])
                        n -= 1
                    except StopIteration:
                        gens.pop(0)

            # ---- lead-in: first batch's QKV + VT, unspliced ----
            lead = [gen_qkv(0, 0), gen_vt(0, 0)]
            pump(lead, 10**6)

            finish = []  # pending output-projection generator chain

            for rep in range(reps):
                for b in range(B):
                    # fillers: next batch's QKV + VT (possibly next rep's)
                    if b + 1 < B:
                        fillers = [gen_qkv(rep, b + 1), gen_vt(rep, b + 1)]
                    elif rep + 1 < reps:
                        fillers = [gen_qkv(rep + 1, 0), gen_vt(rep + 1, 0)]
                    else:
                        fillers = []

                    qsb, ksb = q_sb[b], k_sb[b]
                    vts = vts_all.pop((rep, b))
                    for qb in range(S // QB):
                        qoff = qb * QB
                        cps_h = [
                            cps.tile([65, QB], DT.float32, tag="ctx",
                                     name=f"{rep}_c{b}_{qb}_{h}")
                            for h in range(2)
                        ]
                        for kt in range(NKT):
                            sp = sps.tile([128, 2 * QB], DT.float32, tag="sps",
                                          name=f"{rep}_s{b}_{qb}_{kt}")
                            for h in range(2):
                                hp = slice(h * 64, (h + 1) * 64)
                                nc.tensor.matmul(
                                    sp[:, h * QB : (h + 1) * QB],
                                    ksb[hp, kt * 128 : (kt + 1) * 128],
                                    qsb[hp, qoff : qoff + QB],
                                    start=True, stop=True,
                                )
                            es = esp.tile([128, 2 * QB], DT.bfloat16, tag="es",
                                          name=f"{rep}_e{b}_{qb}_{kt}")
                            ebias = mb[:, b, kt : kt + 1] if with_mask else 0.0
                            # one exp op over both heads halves ACT overhead
                            nc.scalar.activation(
                                es[:], sp[:], AF.Exp, bias=ebias, scale=0.125
                            )
                            for h in range(2):
                                hs = slice(h * QB, (h + 1) * QB)
                                nc.tensor.matmul(
                                    cps_h[h][:],
                                    vts[kt][:, h * 65 : (h + 1) * 65],
                                    es[:, hs],
                                    start=(kt == 0), stop=(kt == NKT - 1),
                                )
                            # splice delayed output projection of the previous
                            # q-block once its normalize chain is surely done,
                            # then QKV/VT fillers for the next batch
                            if kt >= 6 and finish:
                                pump(finish, 1)
                            if kt >= 1:
                                pump(fillers, 2 if kt % 2 else 1)
                        # evict ctx PSUM + start the normalize chain (DVE,
                        # GPSIMD); its PE part is deferred via `finish`
                        ctxn = cnp.tile([128, QB], DT.float32r, tag="cn",
                                        name=f"{rep}_n{b}_{qb}")
                        for h in range(2):
                            cs = smp.tile([65, QB], DT.float32, tag="cs",
                                          name=f"{rep}_cs{b}_{qb}_{h}")
                            nc.vector.tensor_copy(cs[:], cps_h[h][:])
                            rr = smp.tile([1, QB], DT.float32r, tag="rr",
                                          name=f"{rep}_r{b}_{qb}_{h}")
                            with nc.allow_low_precision(reason="softmax recip"):
                                nc.vector.reciprocal(rr[:], cs[64:65, :])
                            bcst = smp.tile([64, QB], DT.float32r, tag="bc",
                                            name=f"{rep}_bb{b}_{qb}_{h}")
                            nc.gpsimd.partition_broadcast(bcst[:], rr[:])
                            with nc.allow_low_precision(reason="ctx normalize"):
                                # all-SBUF multiply -> GPSIMD (DVE evicts PSUM)
                                nc.gpsimd.tensor_mul(
                                    ctxn[h * 64 : (h + 1) * 64, :],
                                    cs[0:64, :], bcst[:],
                                )
                        pump(finish, 10**6)  # at most one qb pending
                        finish = [finish_qb(rep, b, qb, ctxn)]
                    # guarantee next batch's QKV/VT fully emitted before its
                    # attention consumes them (pump budget is normally enough)
                    pump(fillers, 10**6)
            pump(finish, 10**6)
    nc.compile()
    return nc


def _make_runner(nc):
    """jit-compiled shard-mapped executor over the 8 cores, no donation so
    device-resident inputs can be reused across timed calls."""
    import jax
    from jax.experimental.shard_map import shard_map
    from jax.sharding import Mesh, NamedSharding, PartitionSpec

    from concourse import bass2jax as b2j

    b2j.install_neuronx_cc_hook()
    partition_name = nc.partition_id_tensor.name if nc.partition_id_tensor else None
    in_names, out_names, out_avals = [], [], []
    for alloc in nc.m.functions[0].allocations:
        if not isinstance(alloc, mybir.MemoryLocationSet):
            continue
        name = alloc.memorylocations[0].name
        if alloc.kind == "ExternalInput":
            if name != partition_name:
                in_names.append(name)
        elif alloc.kind == "ExternalOutput":
            out_names.append(name)
            out_avals.append(
                jax.core.ShapedArray(tuple(alloc.tensor_shape), DT.np(alloc.dtype))
            )
    n_params = len(in_names)
    all_in_names = list(in_names + out_names)
    if partition_name is not None:
        all_in_names.append(partition_name)

    def _body(*args):
        operands = list(args)
        if partition_name is not None:
            operands.append(b2j.partition_id_tensor())
        outs = b2j._bass_exec_p.bind(
            *operands,
            out_avals=tuple(out_avals),
            in_names=tuple(all_in_names),
            out_names=tuple(out_names),
            lowering_input_output_aliases=(),
            sim_require_finite=True,
            sim_require_nnan=True,
            nc=nc,
        )
        return tuple(outs)

    devices = jax.devices()[:NCORES]
    mesh = Mesh(np.asarray(devices), ("core",))
    spec = PartitionSpec("core")
    n_outs = len(out_names)
    fn = jax.jit(
        shard_map(
            _body,
            mesh=mesh,
            in_specs=(spec,) * (n_params + n_outs),
            out_specs=(spec,) * n_outs,
            check_rep=False,
        ),
        keep_unused=True,
    )

    sharding = NamedSharding(mesh, spec)

    def put(in_maps):
        concat = [
            np.concatenate([np.asarray(m[name]) for m in in_maps], axis=0)
            for name in in_names
        ]
        zeros = [
            np.zeros((NCORES * a.shape[0], *a.shape[1:]), a.dtype) for a in out_avals
        ]
        return [jax.device_put(a, sharding) for a in (*concat, *zeros)]

    return fn, put, out_names, out_avals


def _to_bf16(a):
    import ml_dtypes

    return np.asarray(a, np.float32).astype(ml_dtypes.bfloat16)


def _in_maps(x, attention_mask, Wq, bq, Wk, bk, Wv, bv, Wo, with_mask,
             with_bias=True):
    x = np.ascontiguousarray(np.asarray(x, dtype=np.float32))
    xT = _to_bf16(np.ascontiguousarray(x.reshape(T, D).T))  # (D, T) bf16
    ident = np.eye(128, dtype=np.float32)
    in_maps = []
    for c in range(NCORES):
        r = slice(c * DPC, (c + 1) * DPC)
        m = {
            "xT": xT,
            "wqT": _to_bf16(np.asarray(Wq, np.float32)[r, :].T),
            "wkT": _to_bf16(np.asarray(Wk, np.float32)[r, :].T),
            "wvT": _to_bf16(np.asarray(Wv, np.float32)[r, :].T),
            "woT": np.ascontiguousarray(np.asarray(Wo, np.float32)[:, r].T),
            "ident": _to_bf16(ident),
        }
        if with_bias:
            m["bq"] = np.asarray(bq, np.float32)[r].reshape(DPC, 1)
            m["bk"] = np.asarray(bk, np.float32)[r].reshape(DPC, 1)
            m["bv"] = np.asarray(bv, np.float32)[r].reshape(DPC, 1)
        if with_mask:
            mask = np.asarray(attention_mask)
            mbias = np.where(mask == 0, np.float32(-1e30), np.float32(0.0)).astype(
                np.float32
            )
            m["mbias"] = np.ascontiguousarray(mbias.reshape(B, NKT, 128))
        in_maps.append(m)
    return in_maps


def _prepare(x, attention_mask, Wq, bq, Wk, bk, Wv, bv, Wo, bo):
    """Build (cached), upload inputs, return (fn, dev_args, out_names)."""
    mask = np.asarray(attention_mask)
    with_mask = not bool((mask != 0).all())
    with_bias = bool(
        np.any(np.asarray(bq)) or np.any(np.asarray(bk)) or np.any(np.asarray(bv))
    )
    key = ("runner", with_mask, with_bias)
    if key not in _cache:
        nc = _build(with_mask, with_bias=with_bias)
        _cache[key] = _make_runner(nc)
    fn, put, out_names, out_avals = _cache[key]
    dev_args = put(
        _in_maps(x, attention_mask, Wq, bq, Wk, bk, Wv, bv, Wo, with_mask,
                 with_bias=with_bias)
    )
    return fn, dev_args, out_names


def kernel(x, attention_mask, Wq, bq, Wk, bk, Wv, bv, Wo, bo):
    fn, dev_args, out_names = _prepare(
        x, attention_mask, Wq, bq, Wk, bk, Wv, bv, Wo, bo
    )
    outs = fn(*dev_args)
    out_global = np.asarray(outs[out_names.index("out")]).astype(np.float32)
    acc = out_global.reshape(NCORES, T, D).sum(axis=0, dtype=np.float32)
    acc += np.asarray(bo, np.float32)[None, :]
    return acc.reshape(B, S, D)
